# revision 14
# baseline (speedup 1.0000x reference)
"""Trainium2 Bass kernel for attention pooling over graph segments.

Reference computation (per node i with segment b = batch[i]):
    h   = LN(leaky_relu(feat @ W1 + b1)) * g1 + beta1
    att = exp(h @ W2 + b2)
    s_b = segment_sum(att);  att_n = att / s_b
    out_b = segment_sum(att_n[:, :, None] * feat[:, None, :])   # [B, H, D]
    o = LN(lrelu(out @ W3 + b3)) ; o = LN(lrelu(o @ W4 + b4))

Strategy (8 cores, data parallel by graph):
  - 512 graphs per core, BIN-PACKED on host into 512-node windows of up
    to 8 graphs each (first-fit-decreasing). The packing permutation is
    undone in the host-side gather, so windows need ~no zero padding.
  - A one-hot "window-local slot id" C[n, 0:8] plus a ones column are
    packed after the 128 feat columns (fpx rows, width 137).
  - One merged DRAM stream per core, p-major: per 4096-node batch the
    stream holds [ ft block 128x4096 | fpx block 128x(32*137) ], loaded
    as two contiguous dma_starts (ft / fpx) so the node MLP only waits
    on the ft half.  Batch 0's ft block is split into two dma_starts so
    the first h0 matmul can start after ~0.5 MB.
  - att normalization folds into the output: out_b = (sum att*feat)/s_b.
  - Per 128-node chunk, lhsT = C (x) att (weighted one-hot [128, 8*8],
    built on the otherwise-idle GPSIMD engine) and one matmul against
    [feat | ones] accumulates both sum(att*feat) and s. Two 8-graph
    windows share each PSUM tile via partition-offset matmuls.
  - Window results never touch DRAM: each scaled window-pair tile is
    PE-"transposed" against a PERMUTED identity (so columns land in the
    output MLP's preferred order) and copied with a cheap 2-level AP
    into a persistent SBUF tile acat [128d, (h, w, g)] in bf16; the
    output MLP consumes contiguous <=128-col slices of acat as lhsT.
  - Unified software pipeline, iteration b: loads(b+2) | interleaved
    seg(b-2)/at0(b-1) matmuls (so at0's 128-row LDWEIGHTS hides under
    seg's 129-col streaming) | stats/att/woh(b-1) | h0(b) | finalize |
    output-MLP groups as soon as their windows finalize (so the output
    MLP overlaps the main loop instead of running as a serial tail).
  - h0 runs in [64, 1024] half-PAIR PSUM tiles (2 banks) so Prelu /
    square each fire 4x per batch instead of 8x.

This walrus encodes at most one semaphore wait per instruction;
_split_multiwaits() hoists extra waits onto InstEventSemaphore carriers.

Perf notes: fp32 matmuls stream at 2 cycles/column, bf16 at 1 - all large
matmuls run in bf16 (f32 PSUM accumulation).  All ScalarE functions used
(parametric relu, copy, ln, exp) live in one activation-table set, and
rstd = exp(-0.5*ln(var+eps)) keeps sqrt off the engine, so the ~1.3us
ACT_TABLE_LOAD never recurs (it is also warmed at t=0). fp8 was measured
(numpy emulation) at 3-6e-2 final error vs the 2e-2 budget - rejected.
"""

import os
import sys

import numpy as np

try:  # make concourse importable in bare environments
    import concourse  # noqa: F401
except ImportError:  # pragma: no cover
    sys.path.insert(0, "/opt/trn_rl_repo")

NUM_GRAPHS = 4096
NC_CORES = 8
WG = 8  # graphs per window
D = 128
H = 8
CH = 64  # hidden channels
EPS = 1e-6
SLOPE = 0.01
FPW = D + 1 + WG  # fpx row width: feat | ones | C

# packed f32 constant column offsets (wpk, [128, PKW])
PK_B1 = 0          # [64, 1]
PK_B2 = 1          # [128, 8]  b2 + beta1@W2, broadcast
PKW = 9

LAST_RESULT = None  # BassKernelResults of the most recent run (for test.py)


def _prep_shards(feat, seg, bf16):
    """Bin-pack graphs into 512-node windows per core (<=8 graphs each);
    build the merged p-major stream. Returns slot->graph maps so the host
    gather can undo the packing permutation."""
    G = 32  # chunks per batch
    NW = 512  # nodes per window (4 chunks)
    bounds = np.searchsorted(seg, np.arange(NUM_GRAPHS + 1))
    sizes = np.diff(bounds)
    gpc = NUM_GRAPHS // NC_CORES
    core_bins = []
    maxbins = 0
    for k in range(NC_CORES):
        gs = sizes[k * gpc : (k + 1) * gpc]
        assert gs.max() <= NW
        order = np.argsort(-gs, kind="stable")
        bins = []  # [total, [graph ids]]
        for gi in order:
            sz = int(gs[gi])
            for b in bins:
                if b[0] + sz <= NW and len(b[1]) < WG:
                    b[0] += sz
                    b[1].append(int(gi))
                    break
            else:
                bins.append([sz, [int(gi)]])
        core_bins.append(bins)
        maxbins = max(maxbins, len(bins))
    wpb = G * 128 // NW  # windows per batch (8)
    NWIN = -(-maxbins // wpb) * wpb
    NTOT = NWIN * NW
    NB = NTOT // (G * 128)
    BW = G * 128 + G * FPW
    mg_all = []
    slot_maps = []
    for k in range(NC_CORES):
        fpx = np.zeros((NTOT, FPW), np.float32)
        fpx[:, D] = 1.0  # ones column (harmless on pad rows; C gates them)
        smap = np.full(NWIN * WG, -1, np.int64)
        for j, (_, glist) in enumerate(core_bins[k]):
            cur = j * NW
            for pos, gi in enumerate(glist):
                s = int(bounds[k * gpc + gi])
                e = int(bounds[k * gpc + gi + 1])
                n = e - s
                fpx[cur : cur + n, :D] = feat[s:e]
                fpx[cur + np.arange(n), D + 1 + pos] = 1.0
                smap[j * WG + pos] = k * gpc + gi
                cur += n
        ft = np.ascontiguousarray(fpx[:, :D].T)  # [128, NTOT]
        # merged per-batch stream: [ft block | p-major fpx block]
        ftm = ft.reshape(D, NB, G * 128)
        fxm = np.ascontiguousarray(
            fpx.reshape(NB, G, 128, FPW).transpose(2, 0, 1, 3)
        ).reshape(128, NB, G * FPW)
        mg = np.concatenate([ftm, fxm], axis=2).reshape(128, NB * BW)
        mg_all.append(mg.astype(bf16))
        slot_maps.append(smap)
    return mg_all, NW, NTOT, slot_maps


def _build_program(NW, NTOT, host):
    import concourse.bass as bass
    import concourse.tile as tile
    from concourse import mybir

    f32 = mybir.dt.float32
    bf16 = mybir.dt.bfloat16
    AF = mybir.ActivationFunctionType
    OP = mybir.AluOpType

    NWIN = NTOT // NW  # windows per core (bin-packed)
    NSLOT = NWIN * WG  # graph slots per core (incl. empties)
    CPW = NW // 128  # chunks per window
    NCHUNK = NTOT // 128
    G = 32  # chunks per batch (4096 nodes)
    NB = NCHUNK // G
    BW = G * 128 + G * FPW  # merged stream width per batch
    FPB = G * 128  # fpx block offset within a batch's stream

    use_b2 = bool(np.any(host["b2p"]))
    use_b3 = bool(np.any(host["b3"]))
    use_g3 = not (np.allclose(host["g3"], 1.0) and not np.any(host["be3"]))
    use_b4 = bool(np.any(host["b4"]))
    use_g4 = not (np.allclose(host["g4"], 1.0) and not np.any(host["be4"]))

    NG = -(-NSLOT // 128)  # output slot groups
    GW = NG * 128  # per-head column stride in acat (padded)

    nc = bass.Bass()
    t_mg = nc.declare_dram_parameter("mg", [128, NB * BW], bf16, isOutput=False)
    t_w1b = nc.declare_dram_parameter("w1b", [D, CH], bf16, isOutput=False)
    t_wab = nc.declare_dram_parameter("wab", [128, 10], bf16, isOutput=False)
    t_idb = nc.declare_dram_parameter("idb", [128, 128], bf16, isOutput=False)
    t_wpk = nc.declare_dram_parameter("wpk", [128, PKW], f32, isOutput=False)
    t_w34 = nc.declare_dram_parameter("w34", [128, H * D + D], bf16, isOutput=False)
    t_wvec = nc.declare_dram_parameter("wvec", [128, 6 * D], f32, isOutput=False)
    t_out = nc.declare_dram_parameter("out", [NG * 128, D], f32, isOutput=True)

    with tile.TileContext(nc) as tc:
        with (
            tc.tile_pool(name="consts", bufs=1) as consts,
            tc.tile_pool(name="sb", bufs=5) as sb,
            tc.tile_pool(name="sbm", bufs=6) as sbm,
            tc.tile_pool(name="stats", bufs=6) as stats,
        ):
            # ---- constants (tiles now; DMAs deferred past the first loads) ----
            wpk = consts.tile([128, PKW], f32)
            b1c = wpk[0:CH, PK_B1 : PK_B1 + 1]
            b2bc = wpk[:, PK_B2 : PK_B2 + H]
            w1b = consts.tile([D, CH], bf16)
            wab = consts.tile([128, 10], bf16)
            idb = consts.tile([128, 128], bf16)
            epsc = consts.tile([128, 1], f32)
            nc.vector.memset(epsc, EPS)
            zeroc = consts.tile([128, 1], f32)
            nc.vector.memset(zeroc, 0.0)
            # warm the activation table at t=0 so the ~1.3us ACT_TABLE_LOAD
            # doesn't sit on the critical path of the first batch
            warm = consts.tile([128, 1], f32)
            nc.scalar.activation(warm, zeroc, AF.Exp, bias=zeroc)
            # persistent window results [d, (h, w, g)] in bf16
            acat = consts.tile([128, H * GW], bf16)
            w34 = consts.tile([128, H * D + D], bf16)
            w4sb = w34[:, H * D : H * D + D]
            wvec = consts.tile([128, 6 * D], f32)

            with (
                tc.tile_pool(name="ph0", bufs=2, space=bass.MemorySpace.PSUM) as ph0,
                tc.tile_pool(name="pat", bufs=1, space=bass.MemorySpace.PSUM) as pat,
                tc.tile_pool(name="pm", bufs=2, space=bass.MemorySpace.PSUM) as pm,
                tc.tile_pool(name="pmt", bufs=1, space=bass.MemorySpace.PSUM) as pmt,
                tc.tile_pool(name="wohp", bufs=10) as wohp,
            ):
                m_tiles = {}
                fin_tiles = {}
                woh_tiles = {}
                mt_tiles = {}
                hq_tiles = {}
                at_tiles = {}

                def bc(ap_base, step_g, n_inner, step_inner):
                    return bass.AP(
                        tensor=ap_base.tensor,
                        offset=ap_base.offset,
                        ap=[ap_base.ap[0], [step_g, G], [step_inner, n_inner]],
                    )

                def emit_loads(b, split=False):
                    mtf = sb.tile([128, FPB], bf16, tag="mtf", name=f"mtf{b}",
                                  bufs=4)
                    if split:
                        hw = FPB // 2
                        nc.sync.dma_start(
                            out=mtf[:, 0:hw], in_=t_mg[:, b * BW : b * BW + hw]
                        )
                        nc.sync.dma_start(
                            out=mtf[:, hw:FPB],
                            in_=t_mg[:, b * BW + hw : b * BW + FPB],
                        )
                    else:
                        nc.sync.dma_start(
                            out=mtf, in_=t_mg[:, b * BW : b * BW + FPB]
                        )
                    mtp = sb.tile([128, G * FPW], bf16, tag="mtp", name=f"mtp{b}",
                                  bufs=7)
                    nc.sync.dma_start(
                        out=mtp, in_=t_mg[:, b * BW + FPB : (b + 1) * BW]
                    )
                    mt_tiles[b] = (mtf, mtp)

                def emit_h0(b):
                    # node MLP in 1024-wide half-PAIRS ([64, 1024] spans two
                    # PSUM banks; each 512 half is its own accumulation
                    # group); hq stacks lrelu(h) on partitions 0:64 and its
                    # square on 64:128 so ONE matmul per chunk yields
                    # centered logits + mean + E[h^2]
                    mtf = mt_tiles[b][0]
                    pairs = []
                    for up in range(4):
                        h0p = ph0.tile([64, 1024], f32, tag="h0",
                                       name=f"h0_{b}_{up}")
                        for v in range(2):
                            u = up * 2 + v
                            nc.tensor.matmul(
                                h0p[0:CH, v * 512 : (v + 1) * 512],
                                w1b,
                                mtf[:, u * 512 : (u + 1) * 512],
                                start=True,
                                stop=True,
                                skip_group_check=True,
                            )
                        hq = sbm.tile([128, 1024], bf16, tag="hq",
                                      name=f"hq_{b}_{up}", bufs=12)
                        nc.scalar.activation(
                            hq[0:CH, :], h0p[0:CH, :], AF.Prelu, bias=b1c,
                            scale=1.0, alpha=SLOPE,
                        )
                        sq_eng = nc.gpsimd if up == 3 else nc.vector
                        sq_eng.tensor_mul(
                            hq[CH:128, :], hq[0:CH, :], hq[0:CH, :]
                        )
                        pairs.append(hq)
                    hq_tiles[b] = pairs

                pending_fin = []

                def emit_seg_at0(b):
                    """Interleaved seg matmuls (batch b-3) and at0 matmuls
                    (batch b-2): at0's 128-row LDWEIGHTS hides under seg's
                    129-col streaming, and the deep lag keeps every input
                    (hq, woh) ready a full iteration before its consumer so
                    the list scheduler always has ready PE work. Two 8-graph
                    windows (64 rows of (g,h) each) share one PSUM tile via
                    partition-offset matmuls."""
                    bb = b - 3  # seg batch
                    ba = b - 2  # at0 batch
                    seg_ok = 0 <= bb < NB
                    at_ok = 0 <= ba < NB
                    if seg_ok:
                        mtp = mt_tiles.pop(bb)[1]
                    if at_ok:
                        halves = hq_tiles.pop(ba)
                        at0 = pat.tile([128, G, 12], f32, tag="at0",
                                       name=f"at0_{ba}")
                        at_tiles[ba] = at0
                    for ci in range(G):
                        if seg_ok:
                            wohh = woh_tiles[(bb, ci // 8)]
                            c = bb * G + ci
                            w = c // CPW
                            r = c % CPW
                            wp, lo = w // 2, (w % 2) * 64
                            if r == 0 and lo == 0:
                                m_tiles[wp] = pm.tile(
                                    [128, D + 1], f32, tag="m", name=f"m{wp}"
                                )
                            M = m_tiles[wp]
                            nc.tensor.matmul(
                                M[lo : lo + 64, :],
                                wohh[:, ci % 8, :, :],
                                mtp[:, ci * FPW : ci * FPW + D + 1],
                                start=(r == 0),
                                stop=(r == CPW - 1),
                                skip_group_check=True,
                            )
                            if r == CPW - 1 and lo == 64:
                                pending_fin.append(wp)
                                fin_tiles[wp] = m_tiles.pop(wp)
                        if at_ok:
                            nc.tensor.matmul(
                                at0[:, ci, 0:10],
                                halves[ci // 8][:, (ci % 8) * 128
                                                : (ci % 8) * 128 + 128],
                                wab[:, 0:10],
                                start=True,
                                stop=True,
                            )
                    if seg_ok:
                        woh_tiles.pop((bb, 0))
                        for u in range(1, 4):
                            woh_tiles.pop((bb, u))

                def emit_att(ba):
                    """stats + att + woh for batch ba (consumes at0 emitted in
                    the interleave block of this iteration)."""
                    at0 = at_tiles.pop(ba)
                    mtp = mt_tiles[ba][1]

                    # stats: rstd = exp(-0.5*ln(var+eps)) (no sqrt!)
                    def at0col(col):
                        base = at0[:, 0:G, col : col + 1]
                        return bass.AP(
                            tensor=base.tensor,
                            offset=base.offset,
                            ap=[base.ap[0], [12, G]],
                        )

                    stc = stats.tile([128, G], f32, tag="stc")
                    nc.vector.tensor_copy(stc, at0col(8))
                    st0 = stats.tile([128, G], f32, tag="st0")
                    nc.vector.tensor_mul(st0, stc, stc)
                    stv = stats.tile([128, G], f32, tag="stv")
                    nc.vector.tensor_sub(stv, at0col(9), st0)
                    stl = stats.tile([128, G], f32, tag="stl")
                    nc.scalar.activation(stl, stv, AF.Ln, bias=epsc, scale=1.0)
                    rstd = stats.tile([128, G], f32, tag="rstd")
                    nc.scalar.activation(rstd, stl, AF.Exp, bias=zeroc, scale=-0.5)
                    # wab cols 0..7 hold W2g - colsum(W2g)/CH, so at0 raw is
                    # already mean-centered: att2 = rstd * at0_raw.
                    att = stats.tile([128, G, H], bf16, tag="att")
                    att2 = stats.tile([128, G, H], f32, tag="att2")
                    nc.vector.tensor_mul(
                        att2, at0[:, :, 0:H], bc(rstd[:, 0:1], 1, H, 0)
                    )
                    if use_b2:
                        nc.vector.tensor_add(att2, att2, bc(b2bc[:, 0:1], 0, H, 1))
                    nc.scalar.activation(att, att2, AF.Exp, bias=zeroc)
                    # weighted one-hots for the seg matmuls, in 8-chunk groups
                    # on the otherwise-idle GPSIMD engine
                    for up in range(4):
                        wohh = wohp.tile(
                            [128, 8, WG, H], bf16, tag="woh",
                            name=f"woh_{ba}_{up}", bufs=12,
                        )
                        c_base = mtp[
                            :, up * 8 * FPW + D + 1 : up * 8 * FPW + D + 1 + WG
                        ]
                        a_base = att[:, up * 8 : up * 8 + 8, :]
                        nc.gpsimd.tensor_mul(
                            wohh,
                            bass.AP(
                                tensor=c_base.tensor,
                                offset=c_base.offset,
                                ap=[c_base.ap[0], [FPW, 8], [1, WG], [0, H]],
                            ),
                            bass.AP(
                                tensor=a_base.tensor,
                                offset=a_base.offset,
                                ap=[a_base.ap[0], [H, 8], [0, WG], [1, H]],
                            ),
                        )
                        woh_tiles[(ba, up)] = wohh

                mo_tiles = {}

                def emit_finalize_v():
                    """Scale this iteration's finalized window pairs (emitted
                    right after the seg block so the DVE drains them as each
                    window pair's last seg matmul lands, mid-block)."""
                    for wp in pending_fin:
                        M = fin_tiles.pop(wp)
                        sm = stats.tile([128, 1], f32, tag="sm")
                        nc.vector.tensor_scalar_max(sm, M[:, D : D + 1],
                                                    1e-30)
                        sr_ = stats.tile([128, 1], f32, tag="sr_")
                        nc.vector.reciprocal(sr_, sm)
                        mo = sbm.tile([128, D], bf16, tag="mo")
                        nc.vector.tensor_scalar_mul(mo, M[:, 0:D], sr_)
                        mo_tiles[wp] = mo

                def emit_finalize_pe():
                    """Transpose the scaled pairs against the PERMUTED
                    identity into one shared PSUM tile, then a single
                    3-level-AP copy scatters all four into acat."""
                    if not pending_fin:
                        return
                    assert len(pending_fin) == 4 and (
                        pending_fin[-1] == pending_fin[0] + 3
                    )
                    wp0 = pending_fin[0]
                    moTp = pmt.tile([128, 512], bf16, tag="moT",
                                    name=f"moT{wp0}")
                    for k in range(4):
                        mo = mo_tiles.pop(pending_fin[k])
                        nc.tensor.transpose(
                            moTp[:, k * 128 : (k + 1) * 128], mo, idb
                        )
                    ob = acat[:, wp0 * 2 * WG : wp0 * 2 * WG + 1]
                    ib = moTp[:, 0:1]
                    nc.scalar.copy(
                        bass.AP(
                            tensor=ob.tensor,
                            offset=ob.offset,
                            ap=[ob.ap[0], [GW, H], [1, 8 * WG]],
                        ),
                        bass.AP(
                            tensor=ib.tensor,
                            offset=ib.offset,
                            ap=[ib.ap[0], [2 * WG, H], [128, 4],
                                [1, 2 * WG]],
                        ),
                    )
                    pending_fin.clear()

                def emit_pc(t):
                    # output MLP for slot group t; PSUM rides the h0 ring
                    # (tag-shared slices) and the moT bank for the transpose
                    o2t = ph0.tile([128, 1024], f32, tag="h0", name=f"o2_{t}")
                    o2 = o2t[:, 0:D]
                    for j in range(H):
                        nc.tensor.matmul(
                            o2,
                            acat[:, j * GW + t * 128 : j * GW + (t + 1) * 128],
                            w34[:, j * D : (j + 1) * D],
                            start=(j == 0),
                            stop=(j == H - 1),
                            skip_group_check=True,
                        )
                    o2s = sbm.tile([128, D], f32, tag="o2s", name="o2s")
                    nc.scalar.activation(
                        o2s, o2, AF.Prelu, bias=zeroc, alpha=SLOPE
                    )
                    if use_b3:
                        o2b = sbm.tile([128, D], f32, tag="o2b", name="o2b")
                        nc.vector.tensor_add(o2b, o2, wvec[:, 0:D])
                        nc.vector.scalar_tensor_tensor(
                            o2s, o2b, SLOPE, o2b, OP.mult, OP.max
                        )
                    o2n = _ln_tile(nc, stats, sbm, o2s, "c3", epsc, zeroc, bf16)
                    if use_g3:
                        nc.vector.tensor_mul(o2n, o2n, wvec[:, D : 2 * D])
                        nc.vector.tensor_add(o2n, o2n, wvec[:, 2 * D : 3 * D])
                    oTt = pmt.tile([128, 256], bf16, tag="moT", name=f"oT_{t}")
                    oT = oTt[:, 0:128]
                    nc.tensor.transpose(oT, o2n, idb)
                    oTs = sbm.tile([128, D], bf16, tag="oTs")
                    nc.scalar.copy(oTs, oT)
                    o3t = ph0.tile([128, 1024], f32, tag="h0", name=f"o3_{t}")
                    o3 = o3t[:, 0:D]
                    nc.tensor.matmul(o3, oTs, w4sb, start=True, stop=True,
                                     skip_group_check=True)
                    o3s = sbm.tile([128, D], f32, tag="o3s", name="o3s")
                    nc.scalar.activation(
                        o3s, o3, AF.Prelu, bias=zeroc, alpha=SLOPE
                    )
                    if use_b4:
                        o3b = sbm.tile([128, D], f32, tag="o3b", name="o3b")
                        nc.vector.tensor_add(o3b, o3, wvec[:, 3 * D : 4 * D])
                        nc.vector.scalar_tensor_tensor(
                            o3s, o3b, SLOPE, o3b, OP.mult, OP.max
                        )
                    o3n = _ln_tile(nc, stats, sbm, o3s, "c4", epsc, zeroc, f32)
                    if use_g4:
                        nc.vector.tensor_mul(o3n, o3n, wvec[:, 4 * D : 5 * D])
                        nc.vector.tensor_add(
                            o3n, o3n, wvec[:, 5 * D : 6 * D]
                        )
                    nc.sync.dma_start(
                        out=t_out[t * 128 : (t + 1) * 128, :], in_=o3n
                    )

                # output group t is finalize-ready right after the seg matmuls
                # of the batch holding its last window-pair complete
                pc_ready = {}
                for t in range(NG):
                    last_slot = min((t + 1) * 128, NSLOT) - 1
                    wp = (last_slot // WG) // 2
                    bb = ((2 * wp + 2) * CPW - 1) // G
                    pc_ready.setdefault(bb + 3, []).append(t)

                # ---- prologue DMAs: small weights first, then batch 0 ----
                nc.sync.dma_start(out=w1b, in_=t_w1b[:, :])
                nc.sync.dma_start(out=wab, in_=t_wab[:, :])
                nc.sync.dma_start(out=wpk, in_=t_wpk[:, :])
                emit_loads(0, split=True)
                emit_loads(1)
                nc.sync.dma_start(out=idb, in_=t_idb[:, :])
                nc.sync.dma_start(out=w34, in_=t_w34[:, :])
                if use_b3 or use_g3 or use_b4 or use_g4:
                    nc.sync.dma_start(out=wvec, in_=t_wvec[:, :])

                # ---- unified pipeline; h0 leads each iteration so Prelu /
                # square clear the Scalar/Vector queue heads early, and the
                # finalize scales precede stats so the DVE drains them as
                # window pairs complete mid-block ----
                for b in range(NB + 3):
                    if b + 2 < NB:
                        emit_loads(b + 2)
                    if b < NB:
                        emit_h0(b)
                    emit_seg_at0(b)
                    emit_finalize_v()
                    if 0 <= b - 2 < NB:
                        emit_att(b - 2)
                    emit_finalize_pe()
                    for t in pc_ready.get(b, ()):
                        emit_pc(t)
    return nc


def _ln_tile(nc, stats, sbm, x, tag, epsc, zeroc, out_dtype):
    """LayerNorm along free dim of x [128, D] -> new SBUF tile."""
    from concourse import mybir

    f32 = mybir.dt.float32
    AF = mybir.ActivationFunctionType
    OP = mybir.AluOpType
    bn = stats.tile([128, 6], f32, tag=tag + "bn", name=tag + "bn")
    nc.vector.bn_stats(out=bn, in_=x)
    mv = stats.tile([128, 2], f32, tag=tag + "mv", name=tag + "mv")
    nc.vector.bn_aggr(out=mv, in_=bn)
    sl = stats.tile([128, 1], f32, tag=tag + "sl", name=tag + "sl")
    nc.scalar.activation(sl, mv[:, 1:2], AF.Ln, bias=epsc, scale=1.0)
    rs = stats.tile([128, 1], f32, tag=tag + "rs", name=tag + "rs")
    nc.scalar.activation(rs, sl, AF.Exp, bias=zeroc, scale=-0.5)
    out = sbm.tile([128, x.shape[-1]], out_dtype, tag=tag + "o", name=tag + "o")
    nc.vector.tensor_scalar(out, x, mv[:, 0:1], rs, OP.subtract, OP.mult)
    return out


def _split_multiwaits(nc):
    """Walrus here encodes at most one semaphore wait per instruction; move
    extra waits onto standalone InstEventSemaphore carriers inserted before
    the instruction (same engine stream, so ordering is preserved)."""
    from concourse import mybir

    ctr = 0
    for f in nc.m.functions:
        for blk in f.blocks:
            live = blk.instructions
            snapshot = list(live)
            live.clear()
            for inst in snapshot:
                si = inst.sync_info
                if si is not None and len(si.on_wait) > 1:
                    waits = list(si.on_wait)
                    for w in waits[:-1]:
                        ctr += 1
                        car = mybir.InstEventSemaphore(
                            name=f"WC-{ctr}", ins=[], outs=[]
                        )
                        car.engine = inst.engine
                        car.sync_info = mybir.SyncInfo(on_wait=[w], on_update=[])
                        live.append(car)
                    inst.sync_info = mybir.SyncInfo(
                        on_wait=[waits[-1]], on_update=list(si.on_update)
                    )
                live.append(inst)
    return ctr


def kernel(feat, batch, W1, b1, g1, beta1, W2, b2, W3, b3, g3, beta3, W4, b4,
           g4, beta4):
    global LAST_RESULT
    import ml_dtypes
    from concourse.bass_utils import run_bass_kernel_spmd

    bf16 = ml_dtypes.bfloat16
    feat = np.asarray(feat, np.float32)
    seg = np.asarray(batch).astype(np.int64)
    W1 = np.asarray(W1, np.float32)
    b1 = np.asarray(b1, np.float32)
    g1 = np.asarray(g1, np.float32)
    beta1 = np.asarray(beta1, np.float32)
    W2 = np.asarray(W2, np.float32)
    b2 = np.asarray(b2, np.float32)

    W2g = W2 * g1[:, None]
    b2p = b2 + beta1 @ W2  # [H]

    wab = np.zeros((128, 10), np.float32)
    wab[0:CH, 0:H] = W2g - W2g.sum(axis=0)[None, :] / CH  # fold mean-centering
    wab[0:CH, H] = 1.0 / CH       # mean from the h rows
    wab[CH:128, H + 1] = 1.0 / CH  # E[h^2] from the h^2 rows

    wpk = np.zeros((128, PKW), np.float32)
    wpk[0:CH, PK_B1] = b1
    wpk[CH:128, PK_B1] = b1
    wpk[:, PK_B2 : PK_B2 + H] = b2p[None, :]

    W3m = np.asarray(W3, np.float32).reshape(H, 128, D)  # [j, k, e]
    w34 = np.zeros((128, H * D + D), np.float32)
    w34[:, 0 : H * D] = np.transpose(W3m, (1, 0, 2)).reshape(128, H * D)
    w34[:, H * D : H * D + D] = np.asarray(W4, np.float32)
    wvec = np.zeros((128, 6 * D), np.float32)
    for i, v in enumerate((b3, g3, beta3, b4, g4, beta4)):
        wvec[:, i * D : (i + 1) * D] = np.asarray(v, np.float32)[None, :]

    mg_all, NW, NTOT, slot_maps = _prep_shards(feat, seg, bf16)
    host = {
        "b2p": b2p,
        "b3": np.asarray(b3, np.float32),
        "g3": np.asarray(g3, np.float32),
        "be3": np.asarray(beta3, np.float32),
        "b4": np.asarray(b4, np.float32),
        "g4": np.asarray(g4, np.float32),
        "be4": np.asarray(beta4, np.float32),
    }
    nc = _build_program(NW, NTOT, host)
    _split_multiwaits(nc)

    # column permutation for the window-pair "transpose": the seg matmul
    # leaves rows in (i, g, h) order; acat wants local order (h, i, g).
    # The same permuted identity serves the output-MLP transpose, which
    # permutes that group's output ROWS by pmx; perm[] undoes it below.
    pmx = np.zeros((128, 128), np.float32)
    for i in range(2):
        for g in range(WG):
            for h in range(H):
                pmx[i * 64 + g * H + h, h * 2 * WG + i * WG + g] = 1.0
    perm = np.argmax(pmx, axis=1)  # slot-local row s lands at out row perm[s]

    common = {
        "w1b": W1.astype(bf16),
        "wab": wab.astype(bf16),
        "idb": pmx.astype(bf16),
        "wpk": wpk,
        "w34": w34.astype(bf16),
        "wvec": wvec,
    }
    in_maps = [{"mg": mg_all[k], **common} for k in range(NC_CORES)]
    trace = bool(int(os.environ.get("BASS_KERNEL_TRACE", "0")))
    tmpdir = os.environ.get("BASS_KERNEL_TMPDIR") or None
    res = run_bass_kernel_spmd(
        nc, in_maps, list(range(NC_CORES)), trace=trace, tmpdir=tmpdir
    )
    LAST_RESULT = res
    out = np.zeros((NUM_GRAPHS, D), np.float32)
    for k in range(NC_CORES):
        r = np.asarray(res.results[k]["out"], np.float32)
        m = slot_maps[k]
        for s in np.nonzero(m >= 0)[0]:
            out[m[s]] = r[(s // 128) * 128 + perm[s % 128]]
    return out


# revision 15
# speedup vs baseline: 1.0298x; 1.0298x over previous
"""Trainium2 Bass kernel for attention pooling over graph segments.

Reference computation (per node i with segment b = batch[i]):
    h   = LN(leaky_relu(feat @ W1 + b1)) * g1 + beta1
    att = exp(h @ W2 + b2)
    s_b = segment_sum(att);  att_n = att / s_b
    out_b = segment_sum(att_n[:, :, None] * feat[:, None, :])   # [B, H, D]
    o = LN(lrelu(out @ W3 + b3)) ; o = LN(lrelu(o @ W4 + b4))

Strategy (8 cores, data parallel by graph):
  - 512 graphs per core, BIN-PACKED on host into 512-node windows of up
    to 8 graphs each (first-fit-decreasing). The packing permutation is
    undone in the host-side gather, so windows need ~no zero padding.
  - A one-hot "window-local slot id" C[n, 0:8] plus a ones column are
    packed after the 128 feat columns (fpx rows, width 137).
  - One merged DRAM stream per core, p-major: per 4096-node batch the
    stream holds [ ft block 128x4096 | fpx block 128x(32*137) ], loaded
    as two contiguous dma_starts (ft / fpx) so the node MLP only waits
    on the ft half.  Batch 0's ft block is split into two dma_starts so
    the first h0 matmul can start after ~0.5 MB.
  - att normalization folds into the output: out_b = (sum att*feat)/s_b.
  - Per 128-node chunk, lhsT = C (x) att (weighted one-hot [128, 8*8],
    built on the otherwise-idle GPSIMD engine) and one matmul against
    [feat | ones] accumulates both sum(att*feat) and s. Two 8-graph
    windows share each PSUM tile via partition-offset matmuls.
  - Window results never touch DRAM: each scaled window-pair tile is
    PE-"transposed" against a PERMUTED identity (so columns land in the
    output MLP's preferred order) and copied with a cheap 2-level AP
    into a persistent SBUF tile acat [128d, (h, w, g)] in bf16; the
    output MLP consumes contiguous <=128-col slices of acat as lhsT.
  - Unified software pipeline, iteration b: loads(b+2) | interleaved
    seg(b-2)/at0(b-1) matmuls (so at0's 128-row LDWEIGHTS hides under
    seg's 129-col streaming) | stats/att/woh(b-1) | h0(b) | finalize |
    output-MLP groups as soon as their windows finalize (so the output
    MLP overlaps the main loop instead of running as a serial tail).
  - h0 runs in [64, 1024] half-PAIR PSUM tiles (2 banks) so Prelu /
    square each fire 4x per batch instead of 8x.

This walrus encodes at most one semaphore wait per instruction;
_split_multiwaits() hoists extra waits onto InstEventSemaphore carriers.

Perf notes: fp32 matmuls stream at 2 cycles/column, bf16 at 1 - all large
matmuls run in bf16 (f32 PSUM accumulation).  All ScalarE functions used
(parametric relu, copy, ln, exp) live in one activation-table set, and
rstd = exp(-0.5*ln(var+eps)) keeps sqrt off the engine, so the ~1.3us
ACT_TABLE_LOAD never recurs (it is also warmed at t=0). fp8 was measured
(numpy emulation) at 3-6e-2 final error vs the 2e-2 budget - rejected.
"""

import os
import sys

import numpy as np

try:  # make concourse importable in bare environments
    import concourse  # noqa: F401
except ImportError:  # pragma: no cover
    sys.path.insert(0, "/opt/trn_rl_repo")

NUM_GRAPHS = 4096
NC_CORES = 8
WG = 8  # graphs per window
D = 128
H = 8
CH = 64  # hidden channels
EPS = 1e-6
SLOPE = 0.01
FPW = D + 1 + WG  # fpx row width: feat | ones | C

# packed f32 constant column offsets (wpk, [128, PKW])
PK_B1 = 0          # [64, 1]
PK_B2 = 1          # [128, 8]  b2 + beta1@W2, broadcast
PKW = 9

LAST_RESULT = None  # BassKernelResults of the most recent run (for test.py)


def _prep_shards(feat, seg, bf16):
    """Bin-pack graphs into 512-node windows per core (<=8 graphs each);
    build the merged p-major stream. Returns slot->graph maps so the host
    gather can undo the packing permutation."""
    G = 32  # chunks per batch
    NW = 512  # nodes per window (4 chunks)
    bounds = np.searchsorted(seg, np.arange(NUM_GRAPHS + 1))
    sizes = np.diff(bounds)
    gpc = NUM_GRAPHS // NC_CORES
    core_bins = []
    maxbins = 0
    for k in range(NC_CORES):
        gs = sizes[k * gpc : (k + 1) * gpc]
        assert gs.max() <= NW
        order = np.argsort(-gs, kind="stable")
        bins = []  # [total, [graph ids]]
        for gi in order:
            sz = int(gs[gi])
            for b in bins:
                if b[0] + sz <= NW and len(b[1]) < WG:
                    b[0] += sz
                    b[1].append(int(gi))
                    break
            else:
                bins.append([sz, [int(gi)]])
        core_bins.append(bins)
        maxbins = max(maxbins, len(bins))
    wpb = G * 128 // NW  # windows per batch (8)
    NWIN = -(-maxbins // wpb) * wpb
    NTOT = NWIN * NW
    NB = NTOT // (G * 128)
    BW = G * 128 + G * FPW
    mg_all = []
    slot_maps = []
    for k in range(NC_CORES):
        fpx = np.zeros((NTOT, FPW), np.float32)
        fpx[:, D] = 1.0  # ones column (harmless on pad rows; C gates them)
        smap = np.full(NWIN * WG, -1, np.int64)
        for j, (_, glist) in enumerate(core_bins[k]):
            cur = j * NW
            for pos, gi in enumerate(glist):
                s = int(bounds[k * gpc + gi])
                e = int(bounds[k * gpc + gi + 1])
                n = e - s
                fpx[cur : cur + n, :D] = feat[s:e]
                fpx[cur + np.arange(n), D + 1 + pos] = 1.0
                smap[j * WG + pos] = k * gpc + gi
                cur += n
        ft = np.ascontiguousarray(fpx[:, :D].T)  # [128, NTOT]
        # merged per-batch stream: [ft block | p-major fpx block]
        ftm = ft.reshape(D, NB, G * 128)
        fxm = np.ascontiguousarray(
            fpx.reshape(NB, G, 128, FPW).transpose(2, 0, 1, 3)
        ).reshape(128, NB, G * FPW)
        mg = np.concatenate([ftm, fxm], axis=2).reshape(128, NB * BW)
        mg_all.append(mg.astype(bf16))
        slot_maps.append(smap)
    return mg_all, NW, NTOT, slot_maps


def _build_program(NW, NTOT, host):
    import concourse.bass as bass
    import concourse.tile as tile
    from concourse import mybir

    f32 = mybir.dt.float32
    bf16 = mybir.dt.bfloat16
    AF = mybir.ActivationFunctionType
    OP = mybir.AluOpType

    NWIN = NTOT // NW  # windows per core (bin-packed)
    NSLOT = NWIN * WG  # graph slots per core (incl. empties)
    CPW = NW // 128  # chunks per window
    NCHUNK = NTOT // 128
    G = 32  # chunks per batch (4096 nodes)
    NB = NCHUNK // G
    BW = G * 128 + G * FPW  # merged stream width per batch
    FPB = G * 128  # fpx block offset within a batch's stream

    use_b2 = bool(np.any(host["b2p"]))
    use_b3 = bool(np.any(host["b3"]))
    use_g3 = not (np.allclose(host["g3"], 1.0) and not np.any(host["be3"]))
    use_b4 = bool(np.any(host["b4"]))
    use_g4 = not (np.allclose(host["g4"], 1.0) and not np.any(host["be4"]))

    NG = -(-NSLOT // 128)  # output slot groups
    GW = NG * 128  # per-head column stride in acat (padded)

    nc = bass.Bass()
    t_mg = nc.declare_dram_parameter("mg", [128, NB * BW], bf16, isOutput=False)
    t_w1b = nc.declare_dram_parameter("w1b", [D, CH], bf16, isOutput=False)
    t_wab = nc.declare_dram_parameter("wab", [128, 10], bf16, isOutput=False)
    t_idb = nc.declare_dram_parameter("idb", [128, 128], bf16, isOutput=False)
    t_wpk = nc.declare_dram_parameter("wpk", [128, PKW], f32, isOutput=False)
    t_w34 = nc.declare_dram_parameter("w34", [128, H * D + D], bf16, isOutput=False)
    t_wvec = nc.declare_dram_parameter("wvec", [128, 6 * D], f32, isOutput=False)
    t_out = nc.declare_dram_parameter("out", [NG * 128, D], f32, isOutput=True)

    with tile.TileContext(nc) as tc:
        with (
            tc.tile_pool(name="consts", bufs=1) as consts,
            tc.tile_pool(name="sb", bufs=5) as sb,
            tc.tile_pool(name="sbm", bufs=6) as sbm,
            tc.tile_pool(name="stats", bufs=6) as stats,
        ):
            # ---- constants (tiles now; DMAs deferred past the first loads) ----
            wpk = consts.tile([128, PKW], f32)
            b1c = wpk[0:CH, PK_B1 : PK_B1 + 1]
            b2bc = wpk[:, PK_B2 : PK_B2 + H]
            w1b = consts.tile([D, CH], bf16)
            wab = consts.tile([128, 10], bf16)
            idb = consts.tile([128, 128], bf16)
            epsc = consts.tile([128, 1], f32)
            nc.vector.memset(epsc, EPS)
            zeroc = consts.tile([128, 1], f32)
            nc.vector.memset(zeroc, 0.0)
            # warm the activation table at t=0 so the ~1.3us ACT_TABLE_LOAD
            # doesn't sit on the critical path of the first batch
            warm = consts.tile([128, 1], f32)
            nc.scalar.activation(warm, zeroc, AF.Exp, bias=zeroc)
            # persistent window results [d, (h, w, g)] in bf16
            acat = consts.tile([128, H * GW], bf16)
            w34 = consts.tile([128, H * D + D], bf16)
            w4sb = w34[:, H * D : H * D + D]
            wvec = consts.tile([128, 6 * D], f32)

            with (
                tc.tile_pool(name="ph0", bufs=2, space=bass.MemorySpace.PSUM) as ph0,
                tc.tile_pool(name="pat", bufs=1, space=bass.MemorySpace.PSUM) as pat,
                tc.tile_pool(name="pm", bufs=2, space=bass.MemorySpace.PSUM) as pm,
                tc.tile_pool(name="pmt", bufs=1, space=bass.MemorySpace.PSUM) as pmt,
                tc.tile_pool(name="wohp", bufs=10) as wohp,
            ):
                m_tiles = {}
                fin_tiles = {}
                woh_tiles = {}
                mt_tiles = {}
                hq_tiles = {}
                at_tiles = {}

                def bc(ap_base, step_g, n_inner, step_inner):
                    return bass.AP(
                        tensor=ap_base.tensor,
                        offset=ap_base.offset,
                        ap=[ap_base.ap[0], [step_g, G], [step_inner, n_inner]],
                    )

                def emit_loads(b, split=False):
                    mtf = sb.tile([128, FPB], bf16, tag="mtf", name=f"mtf{b}",
                                  bufs=4)
                    if split:
                        hw = FPB // 2
                        nc.sync.dma_start(
                            out=mtf[:, 0:hw], in_=t_mg[:, b * BW : b * BW + hw]
                        )
                        nc.sync.dma_start(
                            out=mtf[:, hw:FPB],
                            in_=t_mg[:, b * BW + hw : b * BW + FPB],
                        )
                    else:
                        nc.sync.dma_start(
                            out=mtf, in_=t_mg[:, b * BW : b * BW + FPB]
                        )
                    mtp = sb.tile([128, G * FPW], bf16, tag="mtp", name=f"mtp{b}",
                                  bufs=7)
                    nc.sync.dma_start(
                        out=mtp, in_=t_mg[:, b * BW + FPB : (b + 1) * BW]
                    )
                    mt_tiles[b] = (mtf, mtp)

                def emit_h0(b):
                    # node MLP in 1024-wide half-PAIRS ([64, 1024] spans two
                    # PSUM banks; each 512 half is its own accumulation
                    # group); hq stacks lrelu(h) on partitions 0:64 and its
                    # square on 64:128 so ONE matmul per chunk yields
                    # centered logits + mean + E[h^2]
                    mtf = mt_tiles[b][0]
                    pairs = []
                    for up in range(4):
                        h0p = ph0.tile([64, 1024], f32, tag="h0",
                                       name=f"h0_{b}_{up}")
                        for v in range(2):
                            u = up * 2 + v
                            nc.tensor.matmul(
                                h0p[0:CH, v * 512 : (v + 1) * 512],
                                w1b,
                                mtf[:, u * 512 : (u + 1) * 512],
                                start=True,
                                stop=True,
                                skip_group_check=True,
                            )
                        hq = sbm.tile([128, 1024], bf16, tag="hq",
                                      name=f"hq_{b}_{up}", bufs=12)
                        nc.scalar.activation(
                            hq[0:CH, :], h0p[0:CH, :], AF.Prelu, bias=b1c,
                            scale=1.0, alpha=SLOPE,
                        )
                        nc.vector.tensor_mul(
                            hq[CH:128, :], hq[0:CH, :], hq[0:CH, :]
                        )
                        pairs.append(hq)
                    hq_tiles[b] = pairs

                pending_fin = []

                def emit_seg_at0(b):
                    """Interleaved seg matmuls (batch b-3) and at0 matmuls
                    (batch b-2): at0's 128-row LDWEIGHTS hides under seg's
                    129-col streaming, and the deep lag keeps every input
                    (hq, woh) ready a full iteration before its consumer so
                    the list scheduler always has ready PE work. Two 8-graph
                    windows (64 rows of (g,h) each) share one PSUM tile via
                    partition-offset matmuls."""
                    bb = b - 3  # seg batch
                    ba = b - 2  # at0 batch
                    seg_ok = 0 <= bb < NB
                    at_ok = 0 <= ba < NB
                    if seg_ok:
                        mtp = mt_tiles.pop(bb)[1]
                    if at_ok:
                        halves = hq_tiles.pop(ba)
                        at0 = pat.tile([128, G, 12], f32, tag="at0",
                                       name=f"at0_{ba}")
                        at_tiles[ba] = at0
                    for ci in range(G):
                        if seg_ok:
                            wohh = woh_tiles[(bb, ci // 8)]
                            c = bb * G + ci
                            w = c // CPW
                            r = c % CPW
                            wp, lo = w // 2, (w % 2) * 64
                            if r == 0 and lo == 0:
                                m_tiles[wp] = pm.tile(
                                    [128, D + 1], f32, tag="m", name=f"m{wp}"
                                )
                            M = m_tiles[wp]
                            nc.tensor.matmul(
                                M[lo : lo + 64, :],
                                wohh[:, ci % 8, :, :],
                                mtp[:, ci * FPW : ci * FPW + D + 1],
                                start=(r == 0),
                                stop=(r == CPW - 1),
                                skip_group_check=True,
                            )
                            if r == CPW - 1 and lo == 64:
                                pending_fin.append(wp)
                                fin_tiles[wp] = m_tiles.pop(wp)
                        if at_ok:
                            nc.tensor.matmul(
                                at0[:, ci, 0:10],
                                halves[ci // 8][:, (ci % 8) * 128
                                                : (ci % 8) * 128 + 128],
                                wab[:, 0:10],
                                start=True,
                                stop=True,
                            )
                    if seg_ok:
                        woh_tiles.pop((bb, 0))
                        for u in range(1, 4):
                            woh_tiles.pop((bb, u))

                def emit_att(ba):
                    """stats + att + woh for batch ba (consumes at0 emitted in
                    the interleave block of this iteration)."""
                    at0 = at_tiles.pop(ba)
                    mtp = mt_tiles[ba][1]

                    # stats: rstd = exp(-0.5*ln(var+eps)) (no sqrt!)
                    def at0col(col):
                        base = at0[:, 0:G, col : col + 1]
                        return bass.AP(
                            tensor=base.tensor,
                            offset=base.offset,
                            ap=[base.ap[0], [12, G]],
                        )

                    stc = stats.tile([128, G], f32, tag="stc")
                    nc.vector.tensor_copy(stc, at0col(8))
                    st0 = stats.tile([128, G], f32, tag="st0")
                    nc.vector.tensor_mul(st0, stc, stc)
                    stv = stats.tile([128, G], f32, tag="stv")
                    nc.vector.tensor_sub(stv, at0col(9), st0)
                    stl = stats.tile([128, G], f32, tag="stl")
                    nc.scalar.activation(stl, stv, AF.Ln, bias=epsc, scale=1.0)
                    rstd = stats.tile([128, G], f32, tag="rstd")
                    nc.scalar.activation(rstd, stl, AF.Exp, bias=zeroc, scale=-0.5)
                    # wab cols 0..7 hold W2g - colsum(W2g)/CH, so at0 raw is
                    # already mean-centered: att2 = rstd * at0_raw.
                    att = stats.tile([128, G, H], bf16, tag="att")
                    att2 = stats.tile([128, G, H], f32, tag="att2")
                    nc.vector.tensor_mul(
                        att2, at0[:, :, 0:H], bc(rstd[:, 0:1], 1, H, 0)
                    )
                    if use_b2:
                        nc.vector.tensor_add(att2, att2, bc(b2bc[:, 0:1], 0, H, 1))
                    nc.scalar.activation(att, att2, AF.Exp, bias=zeroc)
                    # weighted one-hots for the seg matmuls, in 8-chunk groups
                    # on the otherwise-idle GPSIMD engine
                    for up in range(4):
                        wohh = wohp.tile(
                            [128, 8, WG, H], bf16, tag="woh",
                            name=f"woh_{ba}_{up}", bufs=12,
                        )
                        c_base = mtp[
                            :, up * 8 * FPW + D + 1 : up * 8 * FPW + D + 1 + WG
                        ]
                        a_base = att[:, up * 8 : up * 8 + 8, :]
                        nc.gpsimd.tensor_mul(
                            wohh,
                            bass.AP(
                                tensor=c_base.tensor,
                                offset=c_base.offset,
                                ap=[c_base.ap[0], [FPW, 8], [1, WG], [0, H]],
                            ),
                            bass.AP(
                                tensor=a_base.tensor,
                                offset=a_base.offset,
                                ap=[a_base.ap[0], [H, 8], [0, WG], [1, H]],
                            ),
                        )
                        woh_tiles[(ba, up)] = wohh

                mo_tiles = {}

                def emit_finalize_v():
                    """Scale this iteration's finalized window pairs (emitted
                    right after the seg block so the DVE drains them as each
                    window pair's last seg matmul lands, mid-block)."""
                    for wp in pending_fin:
                        M = fin_tiles.pop(wp)
                        sm = stats.tile([128, 1], f32, tag="sm")
                        nc.vector.tensor_scalar_max(sm, M[:, D : D + 1],
                                                    1e-30)
                        sr_ = stats.tile([128, 1], f32, tag="sr_")
                        nc.vector.reciprocal(sr_, sm)
                        mo = sbm.tile([128, D], bf16, tag="mo")
                        nc.vector.tensor_scalar_mul(mo, M[:, 0:D], sr_)
                        mo_tiles[wp] = mo

                def emit_finalize_pe():
                    """Transpose the scaled pairs against the PERMUTED
                    identity into one shared PSUM tile, then a single
                    3-level-AP copy scatters all four into acat."""
                    if not pending_fin:
                        return
                    assert len(pending_fin) == 4 and (
                        pending_fin[-1] == pending_fin[0] + 3
                    )
                    wp0 = pending_fin[0]
                    moTp = pmt.tile([128, 512], bf16, tag="moT",
                                    name=f"moT{wp0}")
                    for k in range(4):
                        mo = mo_tiles.pop(pending_fin[k])
                        nc.tensor.transpose(
                            moTp[:, k * 128 : (k + 1) * 128], mo, idb
                        )
                    ob = acat[:, wp0 * 2 * WG : wp0 * 2 * WG + 1]
                    ib = moTp[:, 0:1]
                    nc.scalar.copy(
                        bass.AP(
                            tensor=ob.tensor,
                            offset=ob.offset,
                            ap=[ob.ap[0], [GW, H], [1, 8 * WG]],
                        ),
                        bass.AP(
                            tensor=ib.tensor,
                            offset=ib.offset,
                            ap=[ib.ap[0], [2 * WG, H], [128, 4],
                                [1, 2 * WG]],
                        ),
                    )
                    pending_fin.clear()

                def emit_pc(t):
                    # output MLP for slot group t; PSUM rides the h0 ring
                    # (tag-shared slices) and the moT bank for the transpose
                    o2t = ph0.tile([128, 1024], f32, tag="h0", name=f"o2_{t}")
                    o2 = o2t[:, 0:D]
                    for j in range(H):
                        nc.tensor.matmul(
                            o2,
                            acat[:, j * GW + t * 128 : j * GW + (t + 1) * 128],
                            w34[:, j * D : (j + 1) * D],
                            start=(j == 0),
                            stop=(j == H - 1),
                            skip_group_check=True,
                        )
                    o2s = sbm.tile([128, D], f32, tag="o2s", name="o2s")
                    nc.scalar.activation(
                        o2s, o2, AF.Prelu, bias=zeroc, alpha=SLOPE
                    )
                    if use_b3:
                        o2b = sbm.tile([128, D], f32, tag="o2b", name="o2b")
                        nc.vector.tensor_add(o2b, o2, wvec[:, 0:D])
                        nc.vector.scalar_tensor_tensor(
                            o2s, o2b, SLOPE, o2b, OP.mult, OP.max
                        )
                    o2n = _ln_tile(nc, stats, sbm, o2s, "c3", epsc, zeroc, bf16)
                    if use_g3:
                        nc.vector.tensor_mul(o2n, o2n, wvec[:, D : 2 * D])
                        nc.vector.tensor_add(o2n, o2n, wvec[:, 2 * D : 3 * D])
                    oTt = pmt.tile([128, 256], bf16, tag="moT", name=f"oT_{t}")
                    oT = oTt[:, 0:128]
                    nc.tensor.transpose(oT, o2n, idb)
                    oTs = sbm.tile([128, D], bf16, tag="oTs")
                    nc.scalar.copy(oTs, oT)
                    o3t = ph0.tile([128, 1024], f32, tag="h0", name=f"o3_{t}")
                    o3 = o3t[:, 0:D]
                    nc.tensor.matmul(o3, oTs, w4sb, start=True, stop=True,
                                     skip_group_check=True)
                    o3s = sbm.tile([128, D], f32, tag="o3s", name="o3s")
                    nc.scalar.activation(
                        o3s, o3, AF.Prelu, bias=zeroc, alpha=SLOPE
                    )
                    if use_b4:
                        o3b = sbm.tile([128, D], f32, tag="o3b", name="o3b")
                        nc.vector.tensor_add(o3b, o3, wvec[:, 3 * D : 4 * D])
                        nc.vector.scalar_tensor_tensor(
                            o3s, o3b, SLOPE, o3b, OP.mult, OP.max
                        )
                    o3n = _ln_tile(nc, stats, sbm, o3s, "c4", epsc, zeroc, f32)
                    if use_g4:
                        nc.vector.tensor_mul(o3n, o3n, wvec[:, 4 * D : 5 * D])
                        nc.vector.tensor_add(
                            o3n, o3n, wvec[:, 5 * D : 6 * D]
                        )
                    nc.sync.dma_start(
                        out=t_out[t * 128 : (t + 1) * 128, :], in_=o3n
                    )

                # output group t is finalize-ready right after the seg matmuls
                # of the batch holding its last window-pair complete
                pc_ready = {}
                for t in range(NG):
                    last_slot = min((t + 1) * 128, NSLOT) - 1
                    wp = (last_slot // WG) // 2
                    bb = ((2 * wp + 2) * CPW - 1) // G
                    pc_ready.setdefault(bb + 3, []).append(t)

                # ---- prologue DMAs: small weights first, then batch 0 ----
                nc.sync.dma_start(out=w1b, in_=t_w1b[:, :])
                nc.sync.dma_start(out=wab, in_=t_wab[:, :])
                nc.sync.dma_start(out=wpk, in_=t_wpk[:, :])
                emit_loads(0, split=True)
                emit_loads(1)
                nc.sync.dma_start(out=idb, in_=t_idb[:, :])
                nc.sync.dma_start(out=w34, in_=t_w34[:, :])
                if use_b3 or use_g3 or use_b4 or use_g4:
                    nc.sync.dma_start(out=wvec, in_=t_wvec[:, :])

                # ---- unified pipeline; h0 leads each iteration so Prelu /
                # square clear the Scalar/Vector queue heads early, and the
                # finalize scales precede stats so the DVE drains them as
                # window pairs complete mid-block ----
                for b in range(NB + 3):
                    if b + 2 < NB:
                        emit_loads(b + 2)
                    if b < NB:
                        emit_h0(b)
                    emit_seg_at0(b)
                    emit_finalize_v()
                    if 0 <= b - 2 < NB:
                        emit_att(b - 2)
                    emit_finalize_pe()
                    for t in pc_ready.get(b, ()):
                        emit_pc(t)
    return nc


def _ln_tile(nc, stats, sbm, x, tag, epsc, zeroc, out_dtype):
    """LayerNorm along free dim of x [128, D] -> new SBUF tile."""
    from concourse import mybir

    f32 = mybir.dt.float32
    AF = mybir.ActivationFunctionType
    OP = mybir.AluOpType
    bn = stats.tile([128, 6], f32, tag=tag + "bn", name=tag + "bn")
    nc.vector.bn_stats(out=bn, in_=x)
    mv = stats.tile([128, 2], f32, tag=tag + "mv", name=tag + "mv")
    nc.vector.bn_aggr(out=mv, in_=bn)
    sl = stats.tile([128, 1], f32, tag=tag + "sl", name=tag + "sl")
    nc.scalar.activation(sl, mv[:, 1:2], AF.Ln, bias=epsc, scale=1.0)
    rs = stats.tile([128, 1], f32, tag=tag + "rs", name=tag + "rs")
    nc.scalar.activation(rs, sl, AF.Exp, bias=zeroc, scale=-0.5)
    out = sbm.tile([128, x.shape[-1]], out_dtype, tag=tag + "o", name=tag + "o")
    nc.vector.tensor_scalar(out, x, mv[:, 0:1], rs, OP.subtract, OP.mult)
    return out


def _split_multiwaits(nc):
    """Walrus here encodes at most one semaphore wait per instruction; move
    extra waits onto standalone InstEventSemaphore carriers inserted before
    the instruction (same engine stream, so ordering is preserved)."""
    from concourse import mybir

    ctr = 0
    for f in nc.m.functions:
        for blk in f.blocks:
            live = blk.instructions
            snapshot = list(live)
            live.clear()
            for inst in snapshot:
                si = inst.sync_info
                if si is not None and len(si.on_wait) > 1:
                    waits = list(si.on_wait)
                    for w in waits[:-1]:
                        ctr += 1
                        car = mybir.InstEventSemaphore(
                            name=f"WC-{ctr}", ins=[], outs=[]
                        )
                        car.engine = inst.engine
                        car.sync_info = mybir.SyncInfo(on_wait=[w], on_update=[])
                        live.append(car)
                    inst.sync_info = mybir.SyncInfo(
                        on_wait=[waits[-1]], on_update=list(si.on_update)
                    )
                live.append(inst)
    return ctr


def kernel(feat, batch, W1, b1, g1, beta1, W2, b2, W3, b3, g3, beta3, W4, b4,
           g4, beta4):
    global LAST_RESULT
    import ml_dtypes
    from concourse.bass_utils import run_bass_kernel_spmd

    bf16 = ml_dtypes.bfloat16
    feat = np.asarray(feat, np.float32)
    seg = np.asarray(batch).astype(np.int64)
    W1 = np.asarray(W1, np.float32)
    b1 = np.asarray(b1, np.float32)
    g1 = np.asarray(g1, np.float32)
    beta1 = np.asarray(beta1, np.float32)
    W2 = np.asarray(W2, np.float32)
    b2 = np.asarray(b2, np.float32)

    W2g = W2 * g1[:, None]
    b2p = b2 + beta1 @ W2  # [H]

    wab = np.zeros((128, 10), np.float32)
    wab[0:CH, 0:H] = W2g - W2g.sum(axis=0)[None, :] / CH  # fold mean-centering
    wab[0:CH, H] = 1.0 / CH       # mean from the h rows
    wab[CH:128, H + 1] = 1.0 / CH  # E[h^2] from the h^2 rows

    wpk = np.zeros((128, PKW), np.float32)
    wpk[0:CH, PK_B1] = b1
    wpk[CH:128, PK_B1] = b1
    wpk[:, PK_B2 : PK_B2 + H] = b2p[None, :]

    W3m = np.asarray(W3, np.float32).reshape(H, 128, D)  # [j, k, e]
    w34 = np.zeros((128, H * D + D), np.float32)
    w34[:, 0 : H * D] = np.transpose(W3m, (1, 0, 2)).reshape(128, H * D)
    w34[:, H * D : H * D + D] = np.asarray(W4, np.float32)
    wvec = np.zeros((128, 6 * D), np.float32)
    for i, v in enumerate((b3, g3, beta3, b4, g4, beta4)):
        wvec[:, i * D : (i + 1) * D] = np.asarray(v, np.float32)[None, :]

    mg_all, NW, NTOT, slot_maps = _prep_shards(feat, seg, bf16)
    host = {
        "b2p": b2p,
        "b3": np.asarray(b3, np.float32),
        "g3": np.asarray(g3, np.float32),
        "be3": np.asarray(beta3, np.float32),
        "b4": np.asarray(b4, np.float32),
        "g4": np.asarray(g4, np.float32),
        "be4": np.asarray(beta4, np.float32),
    }
    nc = _build_program(NW, NTOT, host)
    _split_multiwaits(nc)

    # column permutation for the window-pair "transpose": the seg matmul
    # leaves rows in (i, g, h) order; acat wants local order (h, i, g).
    # The same permuted identity serves the output-MLP transpose, which
    # permutes that group's output ROWS by pmx; perm[] undoes it below.
    pmx = np.zeros((128, 128), np.float32)
    for i in range(2):
        for g in range(WG):
            for h in range(H):
                pmx[i * 64 + g * H + h, h * 2 * WG + i * WG + g] = 1.0
    perm = np.argmax(pmx, axis=1)  # slot-local row s lands at out row perm[s]

    common = {
        "w1b": W1.astype(bf16),
        "wab": wab.astype(bf16),
        "idb": pmx.astype(bf16),
        "wpk": wpk,
        "w34": w34.astype(bf16),
        "wvec": wvec,
    }
    in_maps = [{"mg": mg_all[k], **common} for k in range(NC_CORES)]
    trace = bool(int(os.environ.get("BASS_KERNEL_TRACE", "0")))
    tmpdir = os.environ.get("BASS_KERNEL_TMPDIR") or None
    res = run_bass_kernel_spmd(
        nc, in_maps, list(range(NC_CORES)), trace=trace, tmpdir=tmpdir
    )
    LAST_RESULT = res
    out = np.zeros((NUM_GRAPHS, D), np.float32)
    for k in range(NC_CORES):
        r = np.asarray(res.results[k]["out"], np.float32)
        m = slot_maps[k]
        for s in np.nonzero(m >= 0)[0]:
            out[m[s]] = r[(s // 128) * 128 + perm[s % 128]]
    return out
